# revision 53
# baseline (speedup 1.0000x reference)
"""Trainium2 Bass kernel for windowed attention with dynamic position bias.

Shapes (hardcoded): qkv [3, 2, 65536, 192], H=W=256, window 8x32 (N=256),
6 heads, head_dim 32. 512 windows total, data-parallel over 8 cores
(64 windows each; each core owns a contiguous band of 64 H-rows of one batch).

Host prep: Q^T/K^T per window in fp16 (no on-device transposes), V extended
with a ones column (denominator trick), bias folded multiplicatively as
E = exp(bias) applied on DVE after the exp.

Per-window device pipeline (software-pipelined, AV delayed one half-step):
  scores S^T[k,q] = K^T.T @ Q^T per (head, k-chunk) fp16 -> PSUM fp32,
  P = exp(scale * S) on ACT -> fp16 SBUF,
  P' = P * E on DVE (fp16, all-SBUF),
  AV: out[q,:] = P'.T @ [V | 1] fp16 -> PSUM fp32,
  DVE reciprocal of ones-column + broadcast multiply, fp16 DMA out.
"""
import sys
import numpy as np

sys.path.insert(0, "/opt/trn_rl_repo")

H_SP, W_SP = 8, 32
NUM_HEADS = 6
DIM = 192
HEAD_DIM = 32
N = H_SP * W_SP          # 256 tokens per window
LN_EPS = 1e-5
SCALE = HEAD_DIM ** -0.5
B, H, W = 2, 256, 256
L = H * W
N_CORES = 8
WINDOWS_PER_CORE = 64    # 8 hb bands x 8 wi
L_PER_CORE = L // 4      # 16384 tokens (64 H-rows)

_BUILT = None


def _np_layer_norm(x, g, b):
    m = x.mean(axis=-1, keepdims=True)
    v = ((x - m) ** 2).mean(axis=-1, keepdims=True)
    return (x - m) / np.sqrt(v + LN_EPS) * g + b


def _host_bias_exp(rpi, rpe_biases, p):
    """DynamicPosBias MLP + gather, producing E = exp(bias) [128, 3072] fp16.

    Column layout: s*256 + q with s = hh*6 + h_local*2 + kk (matches the
    on-device score layout per half hh); rows = k % 128 for chunk kk.
    """
    x = rpe_biases.astype(np.float32)
    pos = x @ p["pos_proj_w"].T + p["pos_proj_b"]
    pos = np.maximum(_np_layer_norm(pos, p["ln1_g"], p["ln1_b"]), 0.0) @ p["fc1_w"].T + p["fc1_b"]
    pos = np.maximum(_np_layer_norm(pos, p["ln2_g"], p["ln2_b"]), 0.0) @ p["fc2_w"].T + p["fc2_b"]
    pos = np.maximum(_np_layer_norm(pos, p["ln3_g"], p["ln3_b"]), 0.0) @ p["fc3_w"].T + p["fc3_b"]
    # pos: [945, 6]; bias[h, q, k] = pos[rpi[q, k], h]
    rel = pos[np.asarray(rpi).reshape(-1)].reshape(N, N, NUM_HEADS)  # [q, k, h]
    E = np.empty((128, 12 * 256), dtype=np.float16)
    for hh in range(2):
        for h_local in range(3):
            h = 3 * hh + h_local
            et = np.exp(rel[:, :, h].T)            # [k, q]
            for kk in range(2):
                s = hh * 6 + h_local * 2 + kk
                E[:, s * 256:(s + 1) * 256] = et[kk * 128:(kk + 1) * 128, :]
    return E


def _build():
    import concourse.bass as bass
    import concourse.mybir as mybir
    import concourse.tile as tile
    from concourse import bacc

    dt = mybir.dt
    nc = bacc.Bacc("TRN2", target_bir_lowering=False, debug=False)
    qkT_in = nc.declare_dram_parameter("qkT_c", [WINDOWS_PER_CORE, 96, 1024], dt.float16, isOutput=False)
    vext_in = nc.declare_dram_parameter("vext_c", [WINDOWS_PER_CORE, 128, 408], dt.float16, isOutput=False)
    E_in = nc.declare_dram_parameter("E_c", [128, 3072], dt.float16, isOutput=False)
    # raw AV accumulator incl. ones-column denominators; normalized on host
    out_c = nc.declare_dram_parameter("out_c", [WINDOWS_PER_CORE, 128, 408], dt.float16, isOutput=True)

    with tile.TileContext(nc) as tc:
        with (
            tc.tile_pool(name="const", bufs=1) as cp,
            tc.tile_pool(name="io", bufs=4) as iop,
            tc.tile_pool(name="work", bufs=3) as wp,
            tc.tile_pool(name="ps_s", bufs=2, space="PSUM") as ps_s,
            tc.tile_pool(name="ps_av", bufs=2, space="PSUM") as ps_av,
        ):
            qkp = vxp = iop
            ptp = ptep = wp
            obp = iop
            # PE p-state warm-up: dummy matmuls (garbage in, overwritten by
            # start=True AV groups later) keep PE busy while first DMAs land,
            # ramping the clock before the first real scores.
            wdm = cp.tile([32, 128], dt.float16, tag="wdm")
            nc.gpsimd.memset(wdm[:], 0.0)
            Et = cp.tile([128, 3072], dt.float16, tag="E")
            nc.gpsimd.dma_start(out=Et[:], in_=E_in[:])

            def load_w(w, qk_eng=nc.sync):
                qk = qkp.tile([96, 1024], dt.float16, tag="qk")
                qk_eng.dma_start(out=qk[:], in_=qkT_in[w])
                vx = vxp.tile([128, 408], dt.float16, tag="vx")
                nc.sync.dma_start(out=vx[:], in_=vext_in[w])
                return qk, vx

            tiles = {0: load_w(0), 1: load_w(1)}
            psas = {}
            ptes = {}

            steps = [(w, hh) for w in range(WINDOWS_PER_CORE) for hh in range(2)]

            def emit_scores(w, hh):
                qk, _ = tiles[w]
                if hh == 0 and w not in psas:
                    psas[w] = ps_av.tile([128, 408], dt.float32, tag="av", name="psa")
                pss = ps_s.tile([128, 1536], dt.float32, tag="s")
                for h_local in range(3):
                    r0 = 32 * h_local
                    for kk in range(2):
                        c0 = (h_local * 2 + kk) * 256
                        nc.tensor.matmul(
                            pss[:, c0:c0 + 256],
                            qk[r0:r0 + 32, (2 + hh) * 256 + kk * 128:(2 + hh) * 256 + kk * 128 + 128],
                            qk[r0:r0 + 32, hh * 256:(hh + 1) * 256],
                            start=True, stop=True, skip_group_check=True)
                pt = ptp.tile([128, 1536], dt.float16, tag="pt")
                nc.scalar.activation(pt[:], pss[:], mybir.ActivationFunctionType.Exp,
                                     scale=float(SCALE))
                pte = ptep.tile([128, 1536], dt.float16, tag="pte")
                nc.vector.tensor_tensor(
                    out=pte[:], in0=pt[:], in1=Et[:, hh * 1536:(hh + 1) * 1536],
                    op=mybir.AluOpType.mult)
                ptes[(w, hh)] = pte

            def emit_av(w, hh):
                pte = ptes.pop((w, hh))
                _, vx = tiles[w]
                psa = psas[w]
                for qc in range(2):
                    for h_local in range(3):
                        h = 3 * hh + h_local
                        for kk in range(2):
                            c0 = (h_local * 2 + kk) * 256 + qc * 128
                            nc.tensor.matmul(
                                psa[:, qc * 204 + h * 34:qc * 204 + (h + 1) * 34],
                                pte[:, c0:c0 + 128],
                                vx[:, kk * 204 + h * 34:kk * 204 + (h + 1) * 34],
                                start=(kk == 0), stop=(kk == 1), skip_group_check=True)
                if hh == 1:
                    # raw accumulator out; normalization happens on host
                    ob = obp.tile([128, 408], dt.float16, tag="ob")
                    # 1/16 scale keeps fp16 in range; cancels in host division
                    nc.vector.tensor_scalar_mul(ob[:], psa[:], 0.0625)
                    (nc.gpsimd if w % 2 == 0 else nc.sync).dma_start(
                        out=out_c[w], in_=ob[:])
                    del psas[w], tiles[w]

            psas[0] = ps_av.tile([128, 408], dt.float32, tag="av", name="psa")
            for _ in range(6):
                nc.tensor.matmul(psas[0][:, 0:128], wdm[:], wdm[:],
                                 start=True, stop=True, skip_group_check=True)

            DELAY = 2
            for i, st in enumerate(steps):
                w, hh = st
                if hh == 0 and w + 2 < WINDOWS_PER_CORE:
                    tiles[w + 2] = load_w(w + 2)
                if i >= DELAY:
                    emit_av(*steps[i - DELAY])
                emit_scores(w, hh)
            for j in range(DELAY, 0, -1):
                emit_av(*steps[-j])
    nc.compile()
    return nc


def _get_nc():
    global _BUILT
    if _BUILT is None:
        _BUILT = _build()
    return _BUILT


def _host_prep_core(qkv, b, row0):
    """Build per-core qkT [64, 96, 1024] fp16 and vext [64, 128, 408] fp16."""
    # windows: w = hb*8 + wi; token = (h, j)
    def im2win_T(x):
        # x: [16384, 192] -> [64, 6 heads, 32 d, 256 q] -> per half [64, 96, 256]
        a = x.reshape(8, 8, 8, 32, NUM_HEADS, HEAD_DIM)      # hb h wi j hd d
        a = a.transpose(0, 2, 4, 5, 1, 3).reshape(64, NUM_HEADS, HEAD_DIM, 256)
        return a
    qT = im2win_T(qkv[0, b, row0:row0 + L_PER_CORE])          # [64, 6, 32, 256]
    kT = im2win_T(qkv[1, b, row0:row0 + L_PER_CORE])
    qkT = np.empty((64, 96, 1024), dtype=np.float16)
    qkT[:, :, 0:256] = qT[:, 0:3].reshape(64, 96, 256)
    qkT[:, :, 256:512] = qT[:, 3:6].reshape(64, 96, 256)
    qkT[:, :, 512:768] = kT[:, 0:3].reshape(64, 96, 256)
    qkT[:, :, 768:1024] = kT[:, 3:6].reshape(64, 96, 256)

    vc = qkv[2, b, row0:row0 + L_PER_CORE].reshape(8, 8, 8, 32, DIM)
    win = vc.transpose(0, 2, 1, 3, 4).reshape(64, 2, 128, NUM_HEADS, HEAD_DIM)
    tmp = np.zeros((64, 2, 128, NUM_HEADS, 34), dtype=np.float16)
    tmp[..., :32] = win
    tmp[..., 32] = 1.0
    vext_c = np.ascontiguousarray(tmp.transpose(0, 2, 1, 3, 4).reshape(64, 128, 408))
    return qkT, vext_c


def kernel(qkv, H, W, rpi, rpe_biases, pos_proj_w, pos_proj_b, ln1_g, ln1_b,
           fc1_w, fc1_b, ln2_g, ln2_b, fc2_w, fc2_b, ln3_g, ln3_b,
           fc3_w, fc3_b, _trace=False):
    from concourse.bass_utils import run_bass_kernel_spmd

    qkv = np.asarray(qkv, dtype=np.float32)
    params = dict(pos_proj_w=pos_proj_w, pos_proj_b=pos_proj_b, ln1_g=ln1_g,
                  ln1_b=ln1_b, fc1_w=fc1_w, fc1_b=fc1_b, ln2_g=ln2_g,
                  ln2_b=ln2_b, fc2_w=fc2_w, fc2_b=fc2_b, ln3_g=ln3_g,
                  ln3_b=ln3_b, fc3_w=fc3_w, fc3_b=fc3_b)
    params = {k: np.asarray(v, dtype=np.float32) for k, v in params.items()}
    E = _host_bias_exp(rpi, rpe_biases, params)

    nc = _get_nc()
    in_maps = []
    for c in range(N_CORES):
        b = c // 4
        row0 = (c % 4) * L_PER_CORE
        qkT_c, vext_c = _host_prep_core(qkv, b, row0)
        in_maps.append({
            "qkT_c": qkT_c,
            "vext_c": vext_c,
            "E_c": E,
        })
    res = run_bass_kernel_spmd(nc, in_maps, list(range(N_CORES)), trace=_trace)
    out = np.empty((B, H, W, DIM), dtype=np.float32)
    for c in range(N_CORES):
        b = c // 4
        h0 = (c % 4) * 64
        o = res.results[c]["out_c"].reshape(64, 128, 2, 6, 34)
        r = o[..., :32] / o[..., 32:33]              # [w, p, qc, hd, c]
        r = r.reshape(8, 8, 4, 32, 2, 6, 32)         # hb wi h4 j qc hd c
        r = r.transpose(0, 4, 2, 1, 3, 5, 6).reshape(64, 256, DIM)
        out[b, h0:h0 + 64, :, :] = r
    if _trace:
        return out, res
    return out
